# revision 13
# baseline (speedup 1.0000x reference)
"""PaiNN (nn_PaiNN_481036337830) Trainium2 Bass kernel, 8 NeuronCores.

Sharding: nodes split 2500/core by dst ownership; each core's nodes are
host-relabeled into bin-packed blocks (<=128 nodes, <=T_FIX*128 edges) so
all cores run one SPMD program over per-core data.  Per interaction:
message MLP in feature-on-partition layout, phi+v rows -> DRAM table
(bf16 768B rows) + AllGather, dma_gather by edge src, per-edge elementwise
(DVE/ACT/GPSIMD), segment-sum over dst via PE matmul with compare-built
one-hot, then the PaiNN update stage.  Block structure is baked at trace
time; compiled fresh per call (cached in-process).
"""
import sys
sys.path.insert(0, "/opt/trn_rl_repo")
import numpy as np
import ml_dtypes

import jax
import concourse.bacc as bacc
import concourse.mybir as mybir
import concourse.tile as tile
from concourse.library_config import mlp as _mlp_lib

F32 = mybir.dt.float32
BF16 = mybir.dt.bfloat16
I16 = mybir.dt.int16
AF = mybir.ActivationFunctionType
OP = mybir.AluOpType

N_NODES = 20000
N_EDGES = 200000
F = 64
NRBF = 20
CUTOFF = 5.0
L = 3
NC_ = 8
NPC = N_NODES // NC_
T_FIX = 10                    # tiles per block
GCHUNK = T_FIX * 128          # gather chunk = one block of edges
PVW = 384                     # pv row: phi[m|l|r](192) + v[x|y|z](192) bf16

_CACHE = {}
import os as _os
_SIM_COMPAT = _os.environ.get("KERNEL_SIM_COMPAT") == "1"


# =====================================================================
# Runner (inlined; reuses compiled executable across calls)
# =====================================================================
class _SpmdRunner:
    def __init__(self, nc, n_cores):
        from jax.sharding import Mesh, PartitionSpec
        from jax.experimental.shard_map import shard_map
        from concourse.bass2jax import (
            _bass_exec_p, install_neuronx_cc_hook, partition_id_tensor)
        install_neuronx_cc_hook()
        self.n_cores = n_cores
        in_names, out_names, out_avals, zero_outs = [], [], [], []
        pname = nc.partition_id_tensor.name if nc.partition_id_tensor else None
        for alloc in nc.m.functions[0].allocations:
            if not isinstance(alloc, mybir.MemoryLocationSet):
                continue
            name = alloc.memorylocations[0].name
            if alloc.kind == "ExternalInput":
                if name != pname:
                    in_names.append(name)
            elif alloc.kind == "ExternalOutput":
                out_names.append(name)
                shape = tuple(alloc.tensor_shape)
                dtype = mybir.dt.np(alloc.dtype)
                out_avals.append(jax.core.ShapedArray(shape, dtype))
                zero_outs.append(np.zeros(shape, dtype))
        self.n_params = len(in_names)
        self.in_names = list(in_names)
        self.out_names, self.out_avals, self.zero_outs = (
            out_names, out_avals, zero_outs)
        all_in = in_names + out_names + ([pname] if pname else [])

        def _body(*args):
            operands = list(args)
            if pname is not None:
                operands.append(partition_id_tensor())
            return tuple(_bass_exec_p.bind(
                *operands, out_avals=tuple(out_avals),
                in_names=tuple(all_in), out_names=tuple(out_names),
                lowering_input_output_aliases=(),
                sim_require_finite=True, sim_require_nnan=True, nc=nc))

        donate = (tuple(range(self.n_params, self.n_params + len(out_names)))
                  if jax.default_backend() == "neuron" else ())
        devices = jax.devices()[:n_cores]
        mesh = Mesh(np.asarray(devices), ("core",))
        in_specs = (PartitionSpec("core"),) * (self.n_params + len(out_names))
        out_specs = (PartitionSpec("core"),) * len(out_names)
        self.fn = jax.jit(
            shard_map(_body, mesh=mesh, in_specs=in_specs,
                      out_specs=out_specs, check_rep=False),
            donate_argnums=donate, keep_unused=True)

    def run(self, in_maps):
        per_core = [[np.asarray(m[n]) for n in self.in_names] for m in in_maps]
        concat_in = [
            np.concatenate([per_core[c][i] for c in range(self.n_cores)], 0)
            for i in range(self.n_params)]
        concat_zeros = [
            np.zeros((self.n_cores * z.shape[0], *z.shape[1:]), z.dtype)
            for z in self.zero_outs]
        outs = self.fn(*concat_in, *concat_zeros)
        outs = [np.asarray(o) for o in outs]
        return [
            {n: outs[i].reshape(self.n_cores, *self.out_avals[i].shape)[c]
             for i, n in enumerate(self.out_names)}
            for c in range(self.n_cores)]


# =====================================================================
# Host preprocessing
# =====================================================================
def _prep(src, dst, eattr):
    packs = []
    for c in range(NC_):
        lo = c * NPC
        sel = (dst // NPC) == c
        es, ed, ea = src[sel], dst[sel], eattr[sel]
        order = np.argsort(ed, kind="stable")
        es, ed, ea = es[order], ed[order], ea[order]
        cnt = np.bincount(ed - lo, minlength=NPC)
        blocks, cur, cure = [], [], 0
        for n in range(NPC):
            cn = int(cnt[n])
            if len(cur) >= 128 or cure + cn > T_FIX * 128:
                blocks.append(cur)
                cur, cure = [], 0
            cur.append(n + lo)
            cure += cn
        if cur:
            blocks.append(cur)
        packs.append((blocks, es, ed, ea, cnt))

    B = max(len(p[0]) for p in packs)
    NR = B * 128
    EP = B * T_FIX * 128

    slot_of = np.full(N_NODES, -1, np.int64)
    blk_of = np.full(N_NODES, -1, np.int64)
    for c, (blocks, *_rest) in enumerate(packs):
        for b, nodes in enumerate(blocks):
            for s_, n in enumerate(nodes):
                slot_of[n] = s_
                blk_of[n] = b
    core_of = np.arange(N_NODES) // NPC
    # pv-row rank (memory order of the AllGathered table): (core, slot, blk)
    rank_pv = core_of * NR + slot_of * B + blk_of
    # master column (block-major): (core, blk, slot)
    col_m = blk_of * 128 + slot_of

    e_src = np.zeros((NC_, EP), np.int64)
    e_slot = np.full((NC_, EP), -1.0, np.float32)
    e_attr = np.zeros((NC_, EP, 3), np.float32)
    e_attr[:, :, 0] = 1.0
    for c, (blocks, es, ed, ea, cnt) in enumerate(packs):
        ptr = 0
        for b, nodes in enumerate(blocks):
            nb = int(sum(int(cnt[n - c * NPC]) for n in nodes))
            base = b * T_FIX * 128
            e_src[c, base:base + nb] = rank_pv[es[ptr:ptr + nb]]
            e_slot[c, base:base + nb] = slot_of[ed[ptr:ptr + nb]]
            e_attr[c, base:base + nb] = ea[ptr:ptr + nb]
            ptr += nb
        assert ptr == len(es)
    return dict(B=B, NR=NR, EP=EP, slot_of=slot_of, blk_of=blk_of,
                col_m=col_m, e_src=e_src, e_slot=e_slot, e_attr=e_attr)


def _wrap_idx(idx):
    n = idx.shape[0]
    return np.tile(idx.astype(np.int16).reshape(n // 16, 16).T, (8, 1))


def _slotmajor(a, n):
    k = a.shape[1]
    return np.ascontiguousarray(
        a.reshape(n // 128, 128, k).transpose(1, 0, 2).reshape(128, -1))


def _pack_weights(inputs, meta):
    f32i, bfi = {}, {}
    qcols = [slice(64, 128), slice(0, 64), slice(128, 192)]  # [m, l, r]
    bfi["ident64"] = np.eye(64, dtype=np.float32)
    f32i["ident128f"] = np.eye(128, dtype=np.float32)
    f32i["c_pi5"] = np.full((128, 1), np.pi / 5.0, np.float32)
    f32i["c_halfpi"] = np.full((128, 1), np.pi / 2.0, np.float32)
    f32i["c_mhalfpi"] = np.full((128, 1), -np.pi / 2.0, np.float32)
    bfi["ident128"] = np.eye(128, dtype=np.float32)
    for it in range(L):
        bfi[f"mW1_{it}"] = np.asarray(inputs["mW1"][it])
        f32i[f"mb1_{it}"] = np.asarray(inputs["mb1"][it])[:, None]
        mW2 = np.asarray(inputs["mW2"][it])
        mb2 = np.asarray(inputs["mb2"][it])
        for q in range(3):
            bfi[f"mW2_{it}_{q}"] = mW2[:, qcols[q]]
            f32i[f"mb2_{it}_{q}"] = mb2[qcols[q]][:, None]
        Wr = np.asarray(inputs["mWr"][it])
        br = np.asarray(inputs["mbr"][it])
        blk = np.zeros((32, 192), np.float32)
        blk[0:NRBF] = Wr
        blk[NRBF] = br
        blk = np.concatenate([blk[:, qcols[q]] for q in range(3)], axis=1)
        bfi[f"Wr4_{it}"] = np.tile(blk, (4, 1))
        bfi[f"uU_{it}"] = np.asarray(inputs["uU"][it]).T
        bfi[f"uV_{it}"] = np.asarray(inputs["uV"][it]).T
        uW1 = np.asarray(inputs["uW1"][it])
        bfi[f"uW1s_{it}"] = uW1[0:F]
        bfi[f"uW1v_{it}"] = uW1[F:2 * F]
        f32i[f"ub1_{it}"] = np.asarray(inputs["ub1"][it])[:, None]
        uW2 = np.asarray(inputs["uW2"][it])
        ub2 = np.asarray(inputs["ub2"][it])
        for q in range(3):
            bfi[f"uW2_{it}_{q}"] = uW2[:, q * F:(q + 1) * F]
            f32i[f"ub2_{it}_{q}"] = ub2[q * F:(q + 1) * F][:, None]

    def blob(items):
        off, cols = {}, 0
        for n, a in items.items():
            r, c = a.shape
            off[n] = (cols, c, r)
            cols += c
        arr = np.zeros((128, cols), np.float32)
        for n, a in items.items():
            c0, c, r = off[n]
            arr[0:r, c0:c0 + c] = a
        return off, arr

    meta["wb_off"], wb = blob(f32i)
    meta["wbh_off"], wbh = blob(bfi)
    meta["wb_cols"] = wb.shape[1]
    meta["wbh_cols"] = wbh.shape[1]
    return wb, wbh.astype(ml_dtypes.bfloat16)


# =====================================================================
# Device program
# =====================================================================
def _build(meta):
    B, NR, EP = meta["B"], meta["NR"], meta["EP"]
    NT = B * T_FIX
    NCHT = (NT + 3) // 4
    NCHK = 512

    nc = bacc.Bacc("TRN2", target_bir_lowering=False, debug=False,
                   num_devices=NC_)

    def din(name, shape, dt=F32):
        return nc.dram_tensor(name, shape, dt, kind="ExternalInput")

    sT_in = din("sT_in", [F, NR])
    vT_in = din("vT_in", [F, 3 * NR])
    vrows_in = din("vrows_in", [128, B * 192], BF16)
    idx_in = din("idx_in", [128, EP // 16], I16)
    eslot_in = din("eslot_in", [128, NT])
    ea_in = din("ea_in", [128, NT * 3])
    iota_in = din("iota_in", [128, 128], BF16)
    wb_in = din("wb_in", [128, meta["wb_cols"]])
    wbh_in = din("wbh_in", [128, meta["wbh_cols"]], BF16)
    s_out = nc.dram_tensor("s_out", [F, NR], F32, kind="ExternalOutput")
    v_out = nc.dram_tensor("v_out", [F, 3 * NR], F32, kind="ExternalOutput")

    wb_off, wbh_off = meta["wb_off"], meta["wbh_off"]

    with tile.TileContext(nc) as tc:
        with (
            tc.tile_pool(name="const", bufs=1) as cp,
            tc.tile_pool(name="state", bufs=1) as st,
            tc.tile_pool(name="work", bufs=2) as wk,
            tc.tile_pool(name="gath", bufs=2) as gt,
            tc.tile_pool(name="pbig", bufs=2, space="PSUM") as pbig,
            tc.tile_pool(name="psm", bufs=2, space="PSUM") as psm,
            tc.tile_pool(name="pagg", bufs=2, space="PSUM") as pagg,
            tc.tile_pool(name="dram", bufs=1, space="DRAM") as dr,
        ):
            nc.gpsimd.load_library(_mlp_lib)

            wbt = cp.tile([128, meta["wb_cols"]], F32)
            nc.sync.dma_start(wbt[:], wb_in[:])
            wbht = cp.tile([128, meta["wbh_cols"]], BF16)
            nc.sync.dma_start(wbht[:], wbh_in[:])
            iota = cp.tile([128, 128], BF16)
            nc.sync.dma_start(iota[:], iota_in[:])
            eslot = cp.tile([128, NT], F32)
            nc.sync.dma_start(eslot[:], eslot_in[:])
            ea = cp.tile([128, NT, 3], F32)
            nc.sync.dma_start(ea[:], ea_in[:].rearrange("p (t c) -> p t c", c=3))
            idx_sb = cp.tile([128, EP // 16], I16)
            nc.sync.dma_start(idx_sb[:], idx_in[:])

            def W(name, dt=F32):
                c0, cols, rows = (wb_off if dt == F32 else wbh_off)[name]
                t = wbt if dt == F32 else wbht
                return t[0:rows, c0:c0 + cols]

            def act_silu(out_ap, in_ap, bias_ap, tag):
                if not _SIM_COMPAT:
                    nc.scalar.activation(out_ap, in_ap, AF.Silu, bias=bias_ap)
                else:
                    p, fr = out_ap.shape[0], out_ap.shape[-1]
                    z = wk.tile([64, NCHK], F32, tag=tag + "z")
                    sg = wk.tile([64, NCHK], F32, tag=tag + "g")
                    nc.scalar.activation(z[0:p, 0:fr], in_ap, AF.Identity,
                                         bias=bias_ap)
                    nc.scalar.activation(sg[0:p, 0:fr], z[0:p, 0:fr],
                                         AF.Sigmoid)
                    nc.vector.tensor_tensor(out_ap, z[0:p, 0:fr],
                                            sg[0:p, 0:fr], OP.mult)

            sT = st.tile([F, NR], F32)
            nc.sync.dma_start(sT[:], sT_in[:])
            vT = st.tile([F, 3, NR], F32)
            nc.sync.dma_start(vT[:], vT_in[:].rearrange("p (c n) -> p c n", c=3))
            sTb = st.tile([F, NR], BF16)
            nc.vector.tensor_copy(sTb[:], sT[:])
            vTb = st.tile([F, 3, NR], BF16)
            nc.vector.tensor_copy(vTb[:], vT[:])

            su_ctx = tc.tile_pool(name="setup", bufs=1)
            su = su_ctx.__enter__()
            # ---- per-edge geometry (one-time) ----
            sq = su.tile([128, NT, 3], F32, tag="geo3")
            nc.vector.tensor_tensor(sq[:], ea[:], ea[:], OP.mult)
            d2 = su.tile([128, NT], F32, tag="geod2")
            nc.vector.tensor_reduce(d2[:], sq[:], axis=mybir.AxisListType.X,
                                    op=OP.add)
            d = st.tile([128, NT], F32)
            nc.scalar.activation(d[:], d2[:], AF.Sqrt)
            rd = st.tile([128, NT], F32)
            nc.vector.reciprocal(rd[:], d[:])
            # clamped theta = min(d,5) * pi/5  in [0, pi]
            dc = su.tile([128, NT], F32, tag="geodc")
            nc.vector.tensor_scalar(dc[:], d[:], float(CUTOFF), None, OP.min)
            s1 = st.tile([128, NT], F32)   # sin(theta)
            nc.scalar.activation(s1[:], dc[:], AF.Sin, scale=W("c_pi5"))
            cneg = su.tile([128, NT], F32, tag="geocn")  # -cos(theta)
            nc.scalar.activation(cneg[:], dc[:], AF.Sin, scale=W("c_pi5"),
                                 bias=W("c_mhalfpi"))
            msk = su.tile([128, NT], F32, tag="geo1b")
            nc.vector.tensor_scalar(msk[:], d[:], float(CUTOFF), None, OP.is_lt)
            fch = st.tile([128, NT], F32)   # 0.25*(cos+1)*mask
            nc.vector.tensor_scalar(fch[:], cneg[:], 1.0, -0.25,
                                    OP.subtract, OP.mult)
            nc.vector.tensor_tensor(fch[:], fch[:], msk[:], OP.mult)
            dirv = st.tile([128, NT, 3], F32)
            rdb = rd[:].unsqueeze(2).broadcast_to([128, NT, 3])
            nc.vector.tensor_tensor(dirv[:], ea[:], rdb, OP.mult)
            rf = st.tile([128, NT], F32)    # (1/d) * fcut = rd * 2 * fch
            nc.vector.tensor_tensor(rf[:], rd[:], fch[:], OP.mult)
            nc.vector.tensor_scalar(rf[:], rf[:], 2.0, None, OP.mult)
            fc2 = st.tile([128, NT], F32)   # fcut
            nc.vector.tensor_scalar(fc2[:], fch[:], 2.0, None, OP.mult)

            # ---- rbf harmonics via Chebyshev recurrence (one-time) ----
            # rbf_all[:, t, k] = sin((k+1) theta);  cols 20=1s,21..31=0 pre-mul
            rbf_all = su.tile([128, NT, 32], F32)
            nc.vector.memset(rbf_all[:], 0.0)
            cn2 = su.tile([128, NT], F32, tag="geocn2")   # -2 cos(theta)
            nc.vector.tensor_scalar(cn2[:], cneg[:], 2.0, None, OP.mult)
            nc.vector.tensor_copy(rbf_all[:, :, 0:1], s1[:].unsqueeze(2))
            tprev = s1
            # t1 = sin(2*theta) = -cn2*s1 - 0 ; build iteratively
            tcur_ap = rbf_all[:, :, 0]
            for k in range(1, NRBF):
                tmpk = su.tile([128, NT], F32, tag="rbfk")
                nc.vector.tensor_tensor(tmpk[:], cn2[:], rbf_all[:, :, k - 1],
                                        OP.mult)
                if k == 1:
                    nc.vector.tensor_scalar(rbf_all[:, :, k], tmpk[:], -1.0,
                                            None, OP.mult)
                else:
                    nc.vector.tensor_tensor(rbf_all[:, :, k], tmpk[:],
                                            rbf_all[:, :, k - 2], OP.add)
                    nc.vector.tensor_scalar(rbf_all[:, :, k], rbf_all[:, :, k],
                                            -1.0, None, OP.mult)
            # scale harmonics by rf, set col 20 = fcut
            rfb32 = rf[:].unsqueeze(2).broadcast_to([128, NT, NRBF])
            nc.vector.tensor_tensor(rbf_all[:, :, 0:NRBF],
                                    rbf_all[:, :, 0:NRBF], rfb32, OP.mult)
            nc.vector.tensor_copy(rbf_all[:, :, NRBF:NRBF + 1],
                                  fc2[:].unsqueeze(2))

            # ---- rbfT: transpose 4-tile chunks -> [128,128] bf16
            rbfT = st.tile([128, NCHT, 128], BF16)
            for ch in range(NCHT):
                t0 = ch * 4
                nt4 = min(4, NT - t0)
                rts = su.tile([128, 128], BF16, tag="rbfb")
                nc.vector.memset(rts[:], 0.0)
                nc.vector.tensor_copy(
                    rts[:, 0:nt4 * 32],
                    rbf_all[:, t0:t0 + nt4, :].rearrange("p a b -> p (a b)"))
                ptr_ = psm.tile([128, 128], BF16, tag="smb")
                nc.tensor.transpose(ptr_[:], rts[:], W("ident128", BF16))
                nc.vector.tensor_copy(rbfT[:, ch, :], ptr_[:])

            su_ctx.__exit__(None, None, None)

            for it in range(L):
                pv_loc = dr.tile([128, B * PVW], BF16, tag=f"pvl{it}")
                pv_all = dr.tile([NC_ * 128, B * PVW], BF16,
                                 addr_space="Shared", tag=f"pva{it}")
                pv_rows = pv_all[:].rearrange("p (b w) -> (p b) w", w=PVW)
                # ---- message MLP ----
                h1 = st.tile([F, NR], BF16, tag=f"h1")
                for n0 in range(0, NR, NCHK):
                    cn = min(NCHK, NR - n0)
                    ph = pbig.tile([64, NCHK], F32, tag="big")
                    nc.tensor.matmul(ph[0:64, 0:cn], W(f"mW1_{it}", BF16),
                                     sTb[:, n0:n0 + cn], start=True, stop=True)
                    act_silu(h1[:, n0:n0 + cn], ph[0:64, 0:cn],
                             W(f"mb1_{it}"), "sl1")
                for b in range(B):
                    n0 = b * 128
                    stg = wk.tile([128, PVW], BF16, tag="pvstg")
                    for q in range(3):
                        pq = pbig.tile([64, NCHK], F32, tag="big")
                        nc.tensor.matmul(pq[0:64, 0:128],
                                         W(f"mW2_{it}_{q}", BF16),
                                         h1[:, n0:n0 + 128], start=True,
                                         stop=True)
                        phiq = wk.tile([64, 128], BF16, tag="phiq")
                        nc.scalar.activation(phiq[:], pq[0:64, 0:128],
                                             AF.Identity,
                                             bias=W(f"mb2_{it}_{q}"))
                        pt = psm.tile([128, 128], BF16, tag="smb")
                        nc.tensor.transpose(pt[0:128, 0:64], phiq[:],
                                            W("ident64", BF16))
                        nc.scalar.activation(stg[:, q * 64:(q + 1) * 64],
                                             pt[0:128, 0:64], AF.Copy)
                    if it == 0:
                        nc.sync.dma_start(stg[:, 192:384],
                                          vrows_in[:, b * 192:(b + 1) * 192])
                    else:
                        for c3 in range(3):
                            vslc = wk.tile([64, 128], BF16, tag="vslc")
                            nc.vector.tensor_copy(vslc[:],
                                                  vTb[:, c3, n0:n0 + 128])
                            pt = psm.tile([128, 128], BF16, tag="smb")
                            nc.tensor.transpose(pt[0:128, 0:64], vslc[:],
                                                W("ident64", BF16))
                            nc.scalar.activation(
                                stg[:, 192 + c3 * 64:256 + c3 * 64],
                                pt[0:128, 0:64], AF.Copy)
                    nc.sync.dma_start(pv_loc[:, b * PVW:(b + 1) * PVW], stg[:])

                nc.gpsimd.collective_compute(
                    "AllGather", OP.bypass,
                    replica_groups=[list(range(NC_))],
                    ins=[pv_loc.opt()], outs=[pv_all.opt()])

                # ---- edge stage ----
                for b in range(B):
                    psA = pagg.tile([128, 448], F32, tag="agg")
                    g = gt.tile([128, T_FIX, PVW], BF16, tag="gg")
                    ch0 = b * GCHUNK
                    nc.gpsimd.dma_gather(
                        g[:], pv_rows,
                        idx_sb[:, ch0 // 16:(ch0 + GCHUNK) // 16],
                        GCHUNK, GCHUNK, PVW, single_packet=False)
                    for tt in range(T_FIX):
                        t = b * T_FIX + tt
                        gg = g[:, tt, :]
                        chw, g4 = t // 4, t % 4
                        pwf = pbig.tile([128, 192], F32, tag="big")
                        nc.tensor.matmul(
                            pwf[:], rbfT[32 * g4:32 * g4 + 32, chw, :],
                            W(f"Wr4_{it}", BF16)[32 * g4:32 * g4 + 32, :],
                            start=True, stop=True,
                            tile_position=(32 * g4, 0))
                        wf = wk.tile([128, 192], BF16, tag="wf")
                        nc.scalar.activation(wf[:], pwf[:], AF.Copy)
                        ed_ = wk.tile([128, 448], BF16, tag="edata")
                        nc.vector.tensor_tensor(ed_[:, 0:64], gg[0:128, 0:64],
                                                wf[:, 0:64], OP.mult)
                        lr_ = wk.tile([128, 128], BF16, tag="lr")
                        nc.vector.tensor_tensor(lr_[:], gg[0:128, 64:192],
                                                wf[:, 64:192], OP.mult)
                        lb = lr_[:, 0:64].unsqueeze(1).broadcast_to(
                            [128, 3, 64])
                        nc.vector.tensor_tensor(
                            ed_[:, 64:256].rearrange("p (c f) -> p c f", c=3),
                            gg[0:128, 192:384].rearrange(
                                "p (c f) -> p c f", c=3),
                            lb, OP.mult)
                        for c3 in range(3):
                            nc.gpsimd.tensor_scalar(
                                ed_[:, 256 + 64 * c3:320 + 64 * c3],
                                lr_[:, 64:128], dirv[:, t, c3:c3 + 1], None,
                                OP.mult)
                        A = wk.tile([128, 128], BF16, tag="Amat")
                        nc.vector.tensor_scalar(A[:], iota[:],
                                                eslot[:, t:t + 1], None,
                                                OP.is_equal)
                        nc.tensor.matmul(psA[:], A[:], ed_[:],
                                         start=(tt == 0),
                                         stop=(tt == T_FIX - 1))
                    n0 = b * 128
                    stg2 = wk.tile([128, 256], F32, tag="dstg")
                    nc.scalar.activation(stg2[:, 0:64], psA[:, 0:64], AF.Copy)
                    nc.scalar.activation(stg2[:, 64:256], psA[:, 64:256],
                                         AF.Copy)
                    nc.vector.tensor_tensor(stg2[:, 64:256], stg2[:, 64:256],
                                            psA[:, 256:448], OP.add)
                    for q in range(4):
                        pt = psm.tile([128, 128], F32, tag="sm")
                        nc.tensor.transpose(pt[0:64, 0:128],
                                            stg2[:, 64 * q:64 * q + 64],
                                            W("ident128f"))
                        if q == 0 and it == 0:
                            nc.vector.tensor_copy(sT[:, n0:n0 + 128],
                                                  pt[0:64, 0:128])
                        elif q == 0:
                            nc.vector.tensor_tensor(
                                sT[:, n0:n0 + 128], sT[:, n0:n0 + 128],
                                pt[0:64, 0:128], OP.add)
                        else:
                            nc.vector.tensor_tensor(
                                vT[:, q - 1, n0:n0 + 128],
                                vT[:, q - 1, n0:n0 + 128],
                                pt[0:64, 0:128], OP.add)
                nc.vector.tensor_copy(sTb[:], sT[:])
                nc.vector.tensor_copy(vTb[:], vT[:])

                # ---- update stage ----
                for n0 in range(0, NR, NCHK):
                    cn = min(NCHK, NR - n0)
                    Uvb = wk.tile([64, 3, NCHK], BF16, tag="Uvb")
                    Vvb = wk.tile([64, 3, NCHK], BF16, tag="Vvb")
                    for c3 in range(3):
                        pu = pbig.tile([64, NCHK], F32, tag="big")
                        nc.tensor.matmul(pu[0:64, 0:cn], W(f"uU_{it}", BF16),
                                         vTb[:, c3, n0:n0 + cn], start=True,
                                         stop=True)
                        nc.vector.tensor_copy(Uvb[:, c3, 0:cn],
                                              pu[0:64, 0:cn])
                        pw2 = pbig.tile([64, NCHK], F32, tag="big")
                        nc.tensor.matmul(pw2[0:64, 0:cn], W(f"uV_{it}", BF16),
                                         vTb[:, c3, n0:n0 + cn], start=True,
                                         stop=True)
                        nc.vector.tensor_copy(Vvb[:, c3, 0:cn],
                                              pw2[0:64, 0:cn])
                    vn = wk.tile([64, NCHK], F32, tag="vnorm")
                    t2_ = wk.tile([64, NCHK], F32, tag="vn2")
                    nc.vector.tensor_tensor(vn[0:64, 0:cn], Vvb[:, 0, 0:cn],
                                            Vvb[:, 0, 0:cn], OP.mult)
                    nc.vector.tensor_tensor(t2_[0:64, 0:cn], Vvb[:, 1, 0:cn],
                                            Vvb[:, 1, 0:cn], OP.mult)
                    nc.vector.tensor_tensor(vn[0:64, 0:cn], vn[0:64, 0:cn],
                                            t2_[0:64, 0:cn], OP.add)
                    nc.vector.tensor_tensor(t2_[0:64, 0:cn], Vvb[:, 2, 0:cn],
                                            Vvb[:, 2, 0:cn], OP.mult)
                    nc.vector.tensor_tensor(vn[0:64, 0:cn], vn[0:64, 0:cn],
                                            t2_[0:64, 0:cn], OP.add)
                    vnb = wk.tile([64, NCHK], BF16, tag="vnb")
                    nc.scalar.activation(vnb[0:64, 0:cn], vn[0:64, 0:cn],
                                         AF.Sqrt)
                    pa = pbig.tile([64, NCHK], F32, tag="big")
                    nc.tensor.matmul(pa[0:64, 0:cn], W(f"uW1s_{it}", BF16),
                                     sTb[:, n0:n0 + cn], start=True,
                                     stop=False)
                    nc.tensor.matmul(pa[0:64, 0:cn], W(f"uW1v_{it}", BF16),
                                     vnb[0:64, 0:cn], start=False, stop=True)
                    hu = wk.tile([64, NCHK], BF16, tag="hu")
                    act_silu(hu[0:64, 0:cn], pa[0:64, 0:cn],
                             W(f"ub1_{it}"), "sl2")
                    aq = []
                    for q in range(3):
                        pq = pbig.tile([64, NCHK], F32, tag="big")
                        nc.tensor.matmul(pq[0:64, 0:cn],
                                         W(f"uW2_{it}_{q}", BF16),
                                         hu[0:64, 0:cn], start=True, stop=True)
                        aqt = wk.tile([64, NCHK], F32, tag=f"aq{q}")
                        nc.scalar.activation(aqt[0:64, 0:cn], pq[0:64, 0:cn],
                                             AF.Identity,
                                             bias=W(f"ub2_{it}_{q}"))
                        aq.append(aqt)
                    S = wk.tile([64, NCHK], F32, tag="Ssum")
                    nc.vector.tensor_tensor(S[0:64, 0:cn], Uvb[:, 0, 0:cn],
                                            Vvb[:, 0, 0:cn], OP.mult)
                    nc.vector.tensor_tensor(t2_[0:64, 0:cn], Uvb[:, 1, 0:cn],
                                            Vvb[:, 1, 0:cn], OP.mult)
                    nc.vector.tensor_tensor(S[0:64, 0:cn], S[0:64, 0:cn],
                                            t2_[0:64, 0:cn], OP.add)
                    nc.vector.tensor_tensor(t2_[0:64, 0:cn], Uvb[:, 2, 0:cn],
                                            Vvb[:, 2, 0:cn], OP.mult)
                    nc.vector.tensor_tensor(S[0:64, 0:cn], S[0:64, 0:cn],
                                            t2_[0:64, 0:cn], OP.add)
                    ds_ = wk.tile([64, NCHK], F32, tag="dsu")
                    nc.vector.tensor_tensor(ds_[0:64, 0:cn], aq[1][0:64, 0:cn],
                                            S[0:64, 0:cn], OP.mult)
                    nc.vector.tensor_tensor(ds_[0:64, 0:cn], ds_[0:64, 0:cn],
                                            aq[2][0:64, 0:cn], OP.add)
                    nc.vector.tensor_tensor(sT[:, n0:n0 + cn],
                                            sT[:, n0:n0 + cn],
                                            ds_[0:64, 0:cn], OP.add)
                    for c3 in range(3):
                        dv_ = wk.tile([64, NCHK], F32, tag="dvu")
                        nc.vector.tensor_tensor(dv_[0:64, 0:cn],
                                                aq[0][0:64, 0:cn],
                                                Uvb[:, c3, 0:cn], OP.mult)
                        nc.vector.tensor_tensor(vT[:, c3, n0:n0 + cn],
                                                vT[:, c3, n0:n0 + cn],
                                                dv_[0:64, 0:cn], OP.add)
                if it < L - 1:
                    nc.vector.tensor_copy(sTb[:], sT[:])
                    nc.vector.tensor_copy(vTb[:], vT[:])

            nc.sync.dma_start(s_out[:], sT[:])
            nc.sync.dma_start(v_out[:], vT[:].rearrange("p c n -> p (c n)"))
    nc.compile()
    return nc


# =====================================================================
# Entry point
# =====================================================================
def kernel(**inputs):
    src = np.asarray(inputs["edge_index"][0]).astype(np.int64)
    dst = np.asarray(inputs["edge_index"][1]).astype(np.int64)
    eattr = np.asarray(inputs["edge_attr"]).astype(np.float32)
    meta = _prep(src, dst, eattr)
    B, NR, EP = meta["B"], meta["NR"], meta["EP"]
    wb, wbh = _pack_weights(inputs, meta)

    s = np.asarray(inputs["s"], np.float32)
    v = np.asarray(inputs["v"], np.float32)
    col_m, blk_of = meta["col_m"], meta["blk_of"]
    core_of = np.arange(N_NODES) // NPC

    iota_b = np.tile(np.arange(128, dtype=np.float32)[None, :], (128, 1)
                     ).astype(ml_dtypes.bfloat16)

    in_maps = []
    for c in range(NC_):
        nid = np.nonzero(core_of == c)[0]
        lr = col_m[nid]
        sT = np.zeros((F, NR), np.float32)
        vT = np.zeros((F, 3, NR), np.float32)
        sT[:, lr] = s[nid].T
        vT[:, :, lr] = v[nid].transpose(1, 2, 0)
        # vrows: slot-major layout [128(slot), B(blk)*192], plane-major rows
        vrows = np.zeros((128, B, 192), np.float32)
        slots = meta["slot_of"][nid]
        blks = blk_of[nid]
        vrows[slots, blks] = v[nid].transpose(0, 2, 1).reshape(len(nid), 192)
        in_maps.append({
            "sT_in": sT, "vT_in": vT.reshape(F, 3 * NR),
            "vrows_in": vrows.reshape(128, B * 192)
                .astype(ml_dtypes.bfloat16),
            "idx_in": _wrap_idx(meta["e_src"][c]),
            "eslot_in": _slotmajor(meta["e_slot"][c][:, None], EP),
            "ea_in": _slotmajor(meta["e_attr"][c], EP),
            "iota_in": iota_b,
            "wb_in": wb, "wbh_in": wbh,
        })

    ck = (B, NR, EP)
    if ck not in _CACHE:
        _CACHE[ck] = _SpmdRunner(_build(meta), NC_)
    res = _CACHE[ck].run(in_maps)

    s_full = np.zeros((N_NODES, F), np.float32)
    v_full = np.zeros((N_NODES, F, 3), np.float32)
    for c in range(NC_):
        nid = np.nonzero(core_of == c)[0]
        lr = col_m[nid]
        s_full[nid] = res[c]["s_out"][:, lr].T
        v_full[nid] = res[c]["v_out"].reshape(F, 3, NR)[:, :, lr]\
            .transpose(2, 0, 1)
    return s_full, v_full
